# revision 1
# baseline (speedup 1.0000x reference)
"""Trainium2 Bass kernel for nn_GeneralizedAttention (Performer-style linear
attention with GELU random features).

Math (per (b,h)):
    qp  = gelu(q @ proj^T)            [n, m]
    kp  = gelu(k @ proj^T)            [n, m]
    ksum= kp.sum(n)                   [m]
    ctx = kp^T @ v                    [m, e]
    den = qp @ ksum                   [n]
    out = (qp @ ctx) / den[:, None]   [n, e]

Sharding: B*H = 64 (b,h) pairs split across 8 cores, 8 pairs each; proj_mat
replicated; no cross-core comms.

On-chip layouts per (b,h):
    q^T, k^T as [128, 16, 128] where partition = (t*64 + d), free = (j, p),
    n = j*256 + t*128 + p.  Both 64-row halves are used, so projection matmuls
    issue in (t=0, t=1) pairs on disjoint PE row groups and overlap.
    qp^T is kept [m, n]-major (feeds the final contraction over m),
    kp is kept [n, m]-major (feeds the context contraction over n).
    The ones column appended to v folds ksum/den into ctx/out as row 64.
"""

import numpy as np

B, H, N, D, M = 4, 16, 4096, 64, 256
NCORES = 8
BH = B * H
BHPC = BH // NCORES  # 8 (b,h) pairs per core
P = 128
NJ = N // 256        # 16 pair-blocks of 256 n
NCH = N // P         # 32 chunks of 128 n
EAUG = D + 1         # 65: e plus the folded ksum/den row


def _emit_body(ctx, tc, out_d, q_d, k_d, v_d, proj_d, bhpc, repeat=1):
    import concourse.bass as bass
    import concourse.mybir as mybir
    from concourse.masks import make_identity

    nc = tc.nc
    f32 = mybir.dt.float32
    bf16 = mybir.dt.bfloat16
    MULT = mybir.AluOpType.mult
    GELU = mybir.ActivationFunctionType.Gelu

    const = ctx.enter_context(tc.tile_pool(name="const", bufs=1))
    inp = ctx.enter_context(tc.tile_pool(name="inp", bufs=6))
    vpool = ctx.enter_context(tc.tile_pool(name="vpool", bufs=3))
    tsb = ctx.enter_context(tc.tile_pool(name="tsb", bufs=6))
    feat = ctx.enter_context(tc.tile_pool(name="feat", bufs=2))
    small = ctx.enter_context(tc.tile_pool(name="small", bufs=3))
    outp = ctx.enter_context(tc.tile_pool(name="outp", bufs=3))
    ps_gen = ctx.enter_context(tc.tile_pool(name="ps_gen", bufs=2, space="PSUM"))
    ps_small = ctx.enter_context(tc.tile_pool(name="ps_small", bufs=2, space="PSUM"))
    ps_acc = ctx.enter_context(tc.tile_pool(name="ps_acc", bufs=2, space="PSUM"))

    ident_bf = const.tile([P, P], bf16, name="ident_bf")
    make_identity(nc, ident_bf)
    ident_f32 = const.tile([P, P], f32, name="ident_f32")
    make_identity(nc, ident_f32)

    # proj^T [d, m] duplicated on both partition halves (rows 0-63 and 64-127)
    proj_nat = const.tile([P, 2, D], f32, name="proj_nat")
    nc.sync.dma_start(proj_nat[:], proj_d.rearrange("(t p) d -> p t d", p=P))
    projT = const.tile([P, M], bf16, name="projT")
    for t in range(2):
        pspt = ps_small.tile([D, P], f32, tag="small", name=f"ps_projT{t}")
        nc.tensor.transpose(pspt[:], proj_nat[:, t, :], ident_f32)
        nc.vector.tensor_copy(projT[0:D, P * t : P * (t + 1)], pspt[:])
        nc.vector.tensor_copy(projT[D:P, P * t : P * (t + 1)], pspt[:])

    if repeat > 1:
        loop_cm = tc.For_i(0, repeat, 1)
        loop_cm.__enter__()

    for bh in range(bhpc):
        # ---- loads (SWDGE cast f32 -> bf16) ----
        q_pairs = inp.tile([P, NJ, P], bf16, tag="qk", name=f"q_pairs{bh}")
        nc.gpsimd.dma_start(
            q_pairs.rearrange("p j (t d) -> p j t d", t=2),
            q_d[bh].rearrange("(j t p) d -> p j t d", t=2, p=P),
        )
        k_pairs = inp.tile([P, NJ, P], bf16, tag="qk", name=f"k_pairs{bh}")
        nc.gpsimd.dma_start(
            k_pairs.rearrange("p j (t d) -> p j t d", t=2),
            k_d[bh].rearrange("(j t p) d -> p j t d", t=2, p=P),
        )
        v_aug = vpool.tile([P, NCH, EAUG], bf16, tag="va", name=f"v_aug{bh}")
        nc.gpsimd.memset(v_aug[:, :, D:EAUG], 1.0)
        nc.gpsimd.dma_start(
            v_aug[:, :, 0:D], v_d[bh].rearrange("(c p) d -> p c d", p=P)
        )

        # ---- transpose q, k into [ (t,d), (j,p) ] ----
        qT = tsb.tile([P, NJ, P], bf16, tag="t", name=f"qT{bh}")
        kT = tsb.tile([P, NJ, P], bf16, tag="t", name=f"kT{bh}")
        for src, dst in ((q_pairs, qT), (k_pairs, kT)):
            for g in range(NJ // 8):
                pst = ps_small.tile([P, 8, P], bf16, tag="small", name=f"ps_t{bh}g{g}")
                for i in range(8):
                    nc.tensor.transpose(pst[:, i, :], src[:, 8 * g + i, :], ident_bf)
                nc.vector.tensor_copy(dst[:, 8 * g : 8 * g + 8, :], pst[:])

        # ---- qp^T = gelu(proj @ q^T)  [m, n] ----
        qpT = feat.tile([P, 2, 2, NJ, P], bf16, tag="qpT", name=f"qpT{bh}")
        for mc in range(2):
            for b4 in range(4):  # j in [4b4, 4b4+4)
                psq = ps_gen.tile([P, 1024], f32, tag="gen", name=f"ps_qp{bh}_{mc}{b4}")
                for t in range(2):
                    # t=0 -> bank A (cols 0:512), t=1 -> bank B: the pair can
                    # stream concurrently on disjoint row groups/banks
                    nc.tensor.matmul(
                        psq[:, 512 * t : 512 * (t + 1)],
                        lhsT=projT[64 * t : 64 * t + 64, P * mc : P * (mc + 1)],
                        rhs=qT[64 * t : 64 * t + 64, 4 * b4 : 4 * b4 + 4, :],
                    )
                nc.scalar.activation(
                    qpT[:, mc, :, 4 * b4 : 4 * b4 + 4, :], psq[:], GELU
                )

        # ---- kp = gelu(k @ proj^T)  [n, m], interleaved with ctx accumulation ----
        kp = feat.tile([P, NCH, M], bf16, tag="kp", name=f"kp{bh}")
        # view with c = 2j + t split so gelu output order (t, jl, m) maps to chunks
        kp_v = kp.rearrange("p (j t) m -> p t j m", t=2)
        ps_ctx = ps_acc.tile([EAUG, M], f32, tag="acc", name=f"ps_ctx{bh}")
        for g in range(8):  # chunks c in [4g, 4g+4)
            psk = ps_gen.tile([P, 1024], f32, tag="gen", name=f"ps_kp{bh}_{g}")
            for jl in range(2):
                j = 2 * g + jl
                for t in range(2):
                    nc.tensor.matmul(
                        psk[:, 256 * (2 * t + jl) : 256 * (2 * t + jl + 1)],
                        lhsT=kT[64 * t : 64 * t + 64, j, :],
                        rhs=projT[64 * t : 64 * t + 64, :],
                    )
            nc.scalar.activation(kp_v[:, :, 2 * g : 2 * g + 2, :], psk[:], GELU)
            for cl in range(4):
                c = 4 * g + cl
                nc.tensor.matmul(
                    ps_ctx[:],
                    lhsT=v_aug[:, c, :],
                    rhs=kp[:, c, :],
                    start=(c == 0),
                    stop=(c == NCH - 1),
                )

        # ---- ctx^T -> ctx_aug [m, e+1] ----
        ctx_sb = small.tile([EAUG, M], bf16, tag="ctxsb", name=f"ctx_sb{bh}")
        nc.vector.tensor_copy(ctx_sb[:], ps_ctx[:])
        ctxT = small.tile([P, 2, EAUG], bf16, tag="ctxT", name=f"ctxT{bh}")
        for mc in range(2):
            psct = ps_small.tile([P, EAUG], bf16, tag="small", name=f"ps_ctxT{bh}{mc}")
            nc.tensor.transpose(
                psct[:],
                ctx_sb[:, P * mc : P * (mc + 1)],
                ident_bf[0:EAUG, 0:EAUG],
            )
            nc.vector.tensor_copy(ctxT[:, mc, :], psct[:])

        # ---- out_un^T = ctx_aug^T @ qp^T (row 64 = den), normalize, transpose ----
        out_stage = outp.tile([P, NJ, 2, D], f32, tag="ost", name=f"out_stage{bh}")
        for t in range(2):
            for jb in range(4):
                psf = ps_acc.tile([EAUG, 512], f32, tag="acc", name=f"ps_fin{bh}{t}{jb}")
                for mc in range(2):
                    nc.tensor.matmul(
                        psf[:],
                        lhsT=ctxT[:, mc, :],
                        rhs=qpT[:, mc, t, 4 * jb : 4 * jb + 4, :],
                        start=(mc == 0),
                        stop=(mc == 1),
                    )
                fin_sb = small.tile([EAUG, 512], f32, tag="fin", name=f"fin_sb{bh}{t}{jb}")
                nc.vector.tensor_copy(fin_sb[:], psf[:])
                psn = ps_small.tile([P, 4, 68], f32, tag="small", name=f"ps_n{bh}{t}{jb}")
                for i in range(4):
                    nc.tensor.transpose(
                        psn[:, i, 0:EAUG],
                        fin_sb[:, P * i : P * (i + 1)],
                        ident_f32[0:EAUG, 0:EAUG],
                    )
                rec = small.tile([P, 4], f32, tag="rec", name=f"rec{bh}{t}{jb}")
                nc.vector.reciprocal(rec[:], psn[:, :, D])
                nc.vector.tensor_tensor(
                    out_stage[:, 4 * jb : 4 * jb + 4, t, :],
                    psn[:, :, 0:D],
                    rec[:, :, None].to_broadcast((P, 4, D)),
                    MULT,
                )
        nc.sync.dma_start(
            out_d[bh].rearrange("(j t p) d -> p j t d", t=2, p=P), out_stage[:]
        )

    if repeat > 1:
        loop_cm.__exit__(None, None, None)


def build(bhpc=BHPC, repeat=1):
    from contextlib import ExitStack

    import concourse.mybir as mybir
    import concourse.tile as tile
    from concourse import bacc

    nc = bacc.Bacc("TRN2", target_bir_lowering=False, debug=False)
    f32 = mybir.dt.float32
    q_d = nc.dram_tensor("q", [bhpc, N, D], f32, kind="ExternalInput").ap()
    k_d = nc.dram_tensor("k", [bhpc, N, D], f32, kind="ExternalInput").ap()
    v_d = nc.dram_tensor("v", [bhpc, N, D], f32, kind="ExternalInput").ap()
    proj_d = nc.dram_tensor("proj_mat", [M, D], f32, kind="ExternalInput").ap()
    out_d = nc.dram_tensor("out", [bhpc, N, D], f32, kind="ExternalOutput").ap()

    with tile.TileContext(nc) as tc:
        with ExitStack() as body_ctx:
            _emit_body(body_ctx, tc, out_d, q_d, k_d, v_d, proj_d, bhpc, repeat)
    nc.compile()
    return nc


_built = None


def _get_built():
    global _built
    if _built is None:
        _built = build()
    return _built


def _shard_inputs(q, k, v, proj_mat):
    qf = np.ascontiguousarray(q.reshape(BH, N, D), dtype=np.float32)
    kf = np.ascontiguousarray(k.reshape(BH, N, D), dtype=np.float32)
    vf = np.ascontiguousarray(v.reshape(BH, N, D), dtype=np.float32)
    pf = np.ascontiguousarray(proj_mat, dtype=np.float32)
    in_maps = []
    for c in range(NCORES):
        s = slice(c * BHPC, (c + 1) * BHPC)
        in_maps.append({"q": qf[s], "k": kf[s], "v": vf[s], "proj_mat": pf})
    return in_maps


def run_on_hw(q, k, v, proj_mat, trace=False, **kwargs):
    from concourse.bass_utils import run_bass_kernel_spmd

    nc = _get_built()
    in_maps = _shard_inputs(q, k, v, proj_mat)
    res = run_bass_kernel_spmd(
        nc, in_maps, core_ids=list(range(NCORES)), trace=trace, **kwargs
    )
    out = np.concatenate([r["out"] for r in res.results], axis=0)
    return out.reshape(B, H, N, D).astype(np.float32), res


def kernel(q, k, v, proj_mat):
    out, _ = run_on_hw(q, k, v, proj_mat, trace=False)
    return out



# revision 2
# speedup vs baseline: 1.9964x; 1.9964x over previous
"""Trainium2 Bass kernel for nn_GeneralizedAttention (Performer-style linear
attention with GELU random features).

Math (per (b,h)):
    qp  = gelu(q @ proj^T)            [n, m]
    kp  = gelu(k @ proj^T)            [n, m]
    ksum= kp.sum(n)                   [m]
    ctx = kp^T @ v                    [m, e]
    den = qp @ ksum                   [n]
    out = (qp @ ctx) / den[:, None]   [n, e]

Sharding: B*H = 64 (b,h) pairs split across 8 cores, 8 pairs each; proj_mat
replicated; no cross-core comms.

v2 design notes (vs v1 baseline at ~274-312us):
  - The final contraction produces out[n, e] DIRECTLY: for each 128-row
    n-chunk c, matmul(lhsT=qpT[:, mc, chunk c] (stationary [m,128]),
    rhs=ctxT[:, mc, :] ([m, e+1])) accumulated over mc.  This removes the
    32 PE output transposes per pair AND the big f32 PSUM->SBUF DVE copies
    (1x mode) that v1 paid; normalization becomes a small strided
    reciprocal + broadcast multiply straight out of PSUM.
  - The ones column appended to v folds ksum/den into ctx/out as row 64.
  - Activations are N=1024 from PSUM (2 banks x 2 bufs), interleaved with
    all other PE work through a software pipeline across the 8 (b,h) pairs
    so ACT (the ~128us/core floor: 16.8M gelu elems at 1 elem/lane/cycle)
    stays saturated while PE runs transposes/ctx/fin of adjacent pairs.
  - q/k chunk layout: n = j*256 + t*128 + p  =>  chunk c = 2j + t covers
    n in [128c, 128c+128); out chunks are therefore n-contiguous.
"""

import numpy as np

B, H, N, D, M = 4, 16, 4096, 64, 256
NCORES = 8
BH = B * H
BHPC = BH // NCORES  # 8 (b,h) pairs per core
P = 128
NJ = N // 256        # 16 pair-blocks of 256 n
NCH = N // P         # 32 chunks of 128 n
EAUG = D + 1         # 65: e plus the folded ksum/den row


def _emit_body(ctx, tc, out_d, q_d, k_d, v_d, proj_d, bhpc, repeat=1):
    import concourse.bass as bass
    import concourse.mybir as mybir
    from concourse.masks import make_identity

    nc = tc.nc
    f32 = mybir.dt.float32
    bf16 = mybir.dt.bfloat16
    MULT = mybir.AluOpType.mult
    GELU = mybir.ActivationFunctionType.Gelu

    const = ctx.enter_context(tc.tile_pool(name="const", bufs=1))
    inp = ctx.enter_context(tc.tile_pool(name="inp", bufs=6))
    vpool = ctx.enter_context(tc.tile_pool(name="vpool", bufs=3))
    tsb = ctx.enter_context(tc.tile_pool(name="tsb", bufs=4))
    feat = ctx.enter_context(tc.tile_pool(name="feat", bufs=2))
    small = ctx.enter_context(tc.tile_pool(name="small", bufs=3))
    outp = ctx.enter_context(tc.tile_pool(name="outp", bufs=2))
    ps_gen = ctx.enter_context(tc.tile_pool(name="ps_gen", bufs=2, space="PSUM"))
    ps_t = ctx.enter_context(tc.tile_pool(name="ps_t", bufs=2, space="PSUM"))
    ps_acc = ctx.enter_context(tc.tile_pool(name="ps_acc", bufs=2, space="PSUM"))

    ident_bf = const.tile([P, P], bf16, name="ident_bf")
    make_identity(nc, ident_bf)
    ident_f32 = const.tile([P, P], f32, name="ident_f32")
    make_identity(nc, ident_f32)

    # proj^T [d, m] duplicated on both partition halves (rows 0-63 and 64-127)
    proj_nat = const.tile([P, 2, D], f32, name="proj_nat")
    nc.sync.dma_start(proj_nat[:], proj_d.rearrange("(t p) d -> p t d", p=P))
    projT = const.tile([P, M], bf16, name="projT")
    for t in range(2):
        pspt = ps_t.tile([D, P], f32, tag="t", name=f"ps_projT{t}")
        nc.tensor.transpose(pspt[:], proj_nat[:, t, :], ident_f32)
        nc.vector.tensor_copy(projT[0:D, P * t : P * (t + 1)], pspt[:])
        nc.vector.tensor_copy(projT[D:P, P * t : P * (t + 1)], pspt[:])

    if repeat > 1:
        loop_cm = tc.For_i(0, repeat, 1)
        loop_cm.__enter__()

    # ---------------- per-(b,h) stage emitters ----------------
    q_tiles = {}
    k_tiles = {}
    v_tiles = {}
    qT_tiles = {}
    kT_tiles = {}
    qpT_tiles = {}
    kp_tiles = {}
    ctxT_tiles = {}

    def emit_loads(bh):
        q_pairs = inp.tile([P, NJ, P], bf16, tag="qk", name=f"q_pairs{bh}")
        nc.gpsimd.dma_start(
            q_pairs.rearrange("p j (t d) -> p j t d", t=2),
            q_d[bh].rearrange("(j t p) d -> p j t d", t=2, p=P),
        )
        k_pairs = inp.tile([P, NJ, P], bf16, tag="qk", name=f"k_pairs{bh}")
        nc.gpsimd.dma_start(
            k_pairs.rearrange("p j (t d) -> p j t d", t=2),
            k_d[bh].rearrange("(j t p) d -> p j t d", t=2, p=P),
        )
        v_aug = vpool.tile([P, NCH, EAUG], bf16, tag="va", name=f"v_aug{bh}")
        nc.gpsimd.memset(v_aug[:, :, D:EAUG], 1.0)
        nc.gpsimd.dma_start(
            v_aug[:, :, 0:D], v_d[bh].rearrange("(c p) d -> p c d", p=P)
        )
        q_tiles[bh], k_tiles[bh], v_tiles[bh] = q_pairs, k_pairs, v_aug

    def emit_transpose(bh, which, g):
        # one burst of 8 PE transposes -> 1 psum bank -> DVE copy out
        if which == "q":
            src = q_tiles[bh]
            if bh not in qT_tiles:
                qT_tiles[bh] = tsb.tile([P, NJ, P], bf16, tag="t", name=f"qT{bh}")
            dst = qT_tiles[bh]
        else:
            src = k_tiles[bh]
            if bh not in kT_tiles:
                kT_tiles[bh] = tsb.tile([P, NJ, P], bf16, tag="t", name=f"kT{bh}")
            dst = kT_tiles[bh]
        pst = ps_t.tile([P, 8, P], bf16, tag="t", name=f"ps_t{bh}{which}{g}")
        for i in range(8):
            nc.tensor.transpose(pst[:, i, :], src[:, 8 * g + i, :], ident_bf)
        nc.vector.tensor_copy(dst[:, 8 * g : 8 * g + 8, :], pst[:])

    def emit_qp_fill(bh, mc, nb):
        # psq [m-half, (t, 4j, p)] = proj @ qT for 4 j-blocks; gelu -> qpT
        qT = qT_tiles[bh]
        if bh not in qpT_tiles:
            # [m, mc, j, t, p]; chunk c = 2j + t
            qpT_tiles[bh] = feat.tile(
                [P, 2, NJ, 2, P], bf16, tag="qpT", name=f"qpT{bh}"
            )
        qpT = qpT_tiles[bh]
        psq = ps_gen.tile([P, 1024], f32, tag="gen", name=f"ps_qp{bh}_{mc}{nb}")
        for t in range(2):
            nc.tensor.matmul(
                psq[:, 512 * t : 512 * (t + 1)],
                lhsT=projT[64 * t : 64 * t + 64, P * mc : P * (mc + 1)],
                rhs=qT[64 * t : 64 * t + 64, 4 * nb : 4 * nb + 4, :],
            )
        dest = qpT[:, mc, 4 * nb : 4 * nb + 4, :, :].rearrange(
            "m j t p -> m t j p"
        )
        nc.scalar.activation(dest, psq[:], GELU)

    def emit_kp_group(bh, g, ctx_ps):
        # 2 j-blocks -> 4 chunks of kp; gelu; then 4 ctx accumulation MMs
        kT = kT_tiles[bh]
        if bh not in kp_tiles:
            kp_tiles[bh] = feat.tile([P, NCH, M], bf16, tag="kp", name=f"kp{bh}")
        kp = kp_tiles[bh]
        psk = ps_gen.tile([P, 1024], f32, tag="gen", name=f"ps_kp{bh}_{g}")
        for jl in range(2):
            j = 2 * g + jl
            for t in range(2):
                # col block (2t + jl): t=0 -> bank A, t=1 -> bank B
                nc.tensor.matmul(
                    psk[:, 256 * (2 * t + jl) : 256 * (2 * t + jl + 1)],
                    lhsT=kT[64 * t : 64 * t + 64, j, :],
                    rhs=projT[64 * t : 64 * t + 64, :],
                )
        # psk col order (t, jl, m); chunk c = 4g + 2jl + t
        dest = kp[:, 4 * g : 4 * g + 4, :].rearrange("p (jl t) m -> p t jl m", t=2)
        nc.scalar.activation(dest, psk[:], GELU)
        v_aug = v_tiles[bh]
        for cl in range(4):
            c = 4 * g + cl
            nc.tensor.matmul(
                ctx_ps[:],
                lhsT=v_aug[:, c, :],
                rhs=kp[:, c, :],
                start=(c == 0),
                stop=(c == NCH - 1),
            )

    def emit_ctx_fin(bh, ctx_ps):
        # ctx [e+1, m] -> bf16 -> transpose to ctxT [m, mc, e+1]
        ctx_sb = small.tile([EAUG, M], bf16, tag="ctxsb", name=f"ctx_sb{bh}")
        nc.vector.tensor_copy(ctx_sb[:], ctx_ps[:])
        ctxT = small.tile([P, 2, EAUG], bf16, tag="ctxT", name=f"ctxT{bh}")
        for mc in range(2):
            psct = ps_t.tile([P, EAUG], bf16, tag="t", name=f"ps_ctxT{bh}{mc}")
            nc.tensor.transpose(
                psct[:],
                ctx_sb[:, P * mc : P * (mc + 1)],
                ident_bf[0:EAUG, 0:EAUG],
            )
            nc.vector.tensor_copy(ctxT[:, mc, :], psct[:])
        ctxT_tiles[bh] = ctxT

    def emit_fin_group(bh, fg, out_stage):
        # 4 chunks: out[n, e] direct; den in col 64
        qpT = qpT_tiles[bh]
        ctxT = ctxT_tiles[bh]
        fin_ps = ps_acc.tile([P, 4, P], f32, tag="acc", name=f"ps_fin{bh}_{fg}")
        for cl in range(4):
            c = 4 * fg + cl
            for mc in range(2):
                nc.tensor.matmul(
                    fin_ps[:, cl, 0:EAUG],
                    lhsT=qpT[:, mc, c // 2, c % 2, :],
                    rhs=ctxT[:, mc, :],
                    start=(mc == 0),
                    stop=(mc == 1),
                )
        rec = small.tile([P, 4], f32, tag="rec", name=f"rec{bh}_{fg}")
        nc.vector.reciprocal(rec[:], fin_ps[:, :, D])
        nc.vector.tensor_tensor(
            out_stage[:, 4 * fg : 4 * fg + 4, :],
            fin_ps[:, :, 0:D],
            rec[:, :, None].to_broadcast((P, 4, D)),
            MULT,
        )

    # ---------------- software pipeline over bh ----------------
    ctx_ps_tiles = {}
    out_stages = {}

    for i in range(bhpc + 1):
        if i < bhpc:
            if i == 0:
                emit_loads(0)
                if bhpc > 1:
                    emit_loads(1)
                for g in range(2):
                    emit_transpose(0, "q", g)
                for g in range(2):
                    emit_transpose(0, "k", g)
            if i + 2 < bhpc:
                emit_loads(i + 2)
            # qp fills for bh=i interleaved with transposes for bh=i+1
            tq = []
            if i + 1 < bhpc:
                tq = [("q", 0), ("q", 1), ("k", 0), ("k", 1)]
            fills = [(mc, nb) for mc in range(2) for nb in range(4)]
            for idx, (mc, nb) in enumerate(fills):
                if idx < len(tq):
                    emit_transpose(i + 1, tq[idx][0], tq[idx][1])
                emit_qp_fill(i, mc, nb)
        # fin for bh=i-1 (overlaps qp acts of bh=i on ACT engine)
        if i >= 1:
            bhp = i - 1
            out_stage = outp.tile([P, NCH, D], f32, tag="ost", name=f"out_stage{bhp}")
            out_stages[bhp] = out_stage
            for fg in range(8):
                emit_fin_group(bhp, fg, out_stage)
            nc.sync.dma_start(
                out_d[bhp].rearrange("(c p) d -> p c d", p=P), out_stage[:]
            )
        if i < bhpc:
            ctx_ps = ps_acc.tile([EAUG, M], f32, tag="acc", name=f"ps_ctx{i}")
            ctx_ps_tiles[i] = ctx_ps
            for g in range(8):
                emit_kp_group(i, g, ctx_ps)
            emit_ctx_fin(i, ctx_ps)

    if repeat > 1:
        loop_cm.__exit__(None, None, None)


def build(bhpc=BHPC, repeat=1):
    from contextlib import ExitStack

    import concourse.mybir as mybir
    import concourse.tile as tile
    from concourse import bacc

    nc = bacc.Bacc("TRN2", target_bir_lowering=False, debug=False)
    f32 = mybir.dt.float32
    q_d = nc.dram_tensor("q", [bhpc, N, D], f32, kind="ExternalInput").ap()
    k_d = nc.dram_tensor("k", [bhpc, N, D], f32, kind="ExternalInput").ap()
    v_d = nc.dram_tensor("v", [bhpc, N, D], f32, kind="ExternalInput").ap()
    proj_d = nc.dram_tensor("proj_mat", [M, D], f32, kind="ExternalInput").ap()
    out_d = nc.dram_tensor("out", [bhpc, N, D], f32, kind="ExternalOutput").ap()

    with tile.TileContext(nc) as tc:
        with ExitStack() as body_ctx:
            _emit_body(body_ctx, tc, out_d, q_d, k_d, v_d, proj_d, bhpc, repeat)
    nc.compile()
    return nc


_built = None


def _get_built():
    global _built
    if _built is None:
        _built = build()
    return _built


def _shard_inputs(q, k, v, proj_mat):
    qf = np.ascontiguousarray(q.reshape(BH, N, D), dtype=np.float32)
    kf = np.ascontiguousarray(k.reshape(BH, N, D), dtype=np.float32)
    vf = np.ascontiguousarray(v.reshape(BH, N, D), dtype=np.float32)
    pf = np.ascontiguousarray(proj_mat, dtype=np.float32)
    in_maps = []
    for c in range(NCORES):
        s = slice(c * BHPC, (c + 1) * BHPC)
        in_maps.append({"q": qf[s], "k": kf[s], "v": vf[s], "proj_mat": pf})
    return in_maps


def run_on_hw(q, k, v, proj_mat, trace=False, **kwargs):
    from concourse.bass_utils import run_bass_kernel_spmd

    nc = _get_built()
    in_maps = _shard_inputs(q, k, v, proj_mat)
    res = run_bass_kernel_spmd(
        nc, in_maps, core_ids=list(range(NCORES)), trace=trace, **kwargs
    )
    out = np.concatenate([r["out"] for r in res.results], axis=0)
    return out.reshape(B, H, N, D).astype(np.float32), res


def kernel(q, k, v, proj_mat):
    out, _ = run_on_hw(q, k, v, proj_mat, trace=False)
    return out
